# revision 18
# baseline (speedup 1.0000x reference)
"""Masked dot-product attention (B=16, Lq=Lk=2048, d=64) on 8 TRN2 NeuronCores.

Distribution
------------
Attention rows are independent, so work is split into 64 units = (batch,
512-query chunk). Unit cost = ceil(valid_len/128) k-tiles; fully-masked
k-tiles contribute exactly zero and are skipped. Units are sorted by cost
(ascending) and snake-assigned to 8 slots x 8 cores; each slot's tile
count is the max within the slot, so all 8 cores run ONE shared SPMD
program (per-core differences live only in the staged data).

Device math per unit (S^T formulation; softmax over the partition axis):
    s_t[k, q]  = (K^T_t weights) @ Q^T           (PE, bf16 x bf16, PSUM)
    p_t[k, q]  = exp(0.125 * s_t)                (ACT, PSUM->SBUF, bf16)
    pv[v, q]  += V'_t^T @ p_t                    (PE, accumulate over t)
where V'_t = [V rows | ones], with rows >= valid_len zeroed on the host —
this applies the key mask AND computes the softmax denominator l = pv[64]
inside the same matmul. The normalize o = pv[0:64] / pv[64] runs on the
HOST during the gather (device ships raw pv per slot) — no on-device
reciprocal/broadcast chain, so the kernel tail is one copy + one DMA.

v2 layout/schedule changes vs the 62us baseline (trace-driven):
- kv is staged in bf16, PAIR-PACKED: per pair of k-tiles the staged
  region is [V_even(65) | V_odd(65) | K_pair(128)] = 258 bf16 cols,
  where K_pair holds K^T of the even tile in partitions 0-63 and of the
  odd tile in partitions 64-127. This removes the half-partition zero
  padding of the old layout AND halves the bytes: 8.1MB -> 2.9MB HBM
  per core (the old kv stream saturated ~360GB/s for 16us).
- S matmuls are emitted in strictly adjacent (even, odd) pairs that
  occupy disjoint PE array halves; PV batches are only injected at pair
  boundaries, so every pair streams concurrently (~427ns for 2 tiles).
  exp instructions (ACT queue) may land mid-pair — they don't break PE
  queue adjacency.
- Q^T is staged doubled into both partition halves (bf16); slot 0's
  chunk is DMA'd first, slots 1-7 follow in one 0.9MB prefetch.
- All DMAs are dispatched from the Sync queue (HWDGE); ~620ns per
  dispatch, 25 dispatches total.
- Every TPB instruction may carry at most ONE sync wait on this walrus;
  split_multi_waits() post-processes the scheduled program.
"""
import numpy as np

import concourse.bass as bass
import concourse.mybir as mybir
import concourse.tile as tile
from concourse.bass_utils import run_bass_kernel_spmd


def _ensure_ntff_hook():
    """The agent image's `antenv` package lacks `axon_hooks`, so
    run_bass_kernel_spmd(trace=True) dies importing it (trn_boot degrades
    silently). Inject a live module with the get/set pair and register the
    ctypes-driven hook trn_boot would have. No-op if already present."""
    import importlib.util
    import sys
    import types
    try:
        if importlib.util.find_spec("antenv.axon_hooks") is not None:
            return
    except Exception:
        pass
    if "antenv.axon_hooks" in sys.modules:
        return
    mod = types.ModuleType("antenv.axon_hooks")
    mod._hook = None
    mod.set_axon_ntff_profile_hook = lambda h: setattr(mod, "_hook", h)
    mod.get_axon_ntff_profile_hook = lambda: mod._hook
    sys.modules["antenv.axon_hooks"] = mod
    try:
        import antenv
        antenv.axon_hooks = mod
        from trn_agent_boot.trn_boot import _ntff_profile_via_ctypes
        mod._hook = _ntff_profile_via_ctypes("/opt/axon/libaxon_pjrt.so")
    except Exception:
        pass


_ensure_ntff_hook()


def split_multi_waits(nc):
    """TRN2 TPB instructions encode a single sync-wait slot. Tile's
    add_semaphores can emit several waits on one instruction (and the
    kernel-tail drain aggregates one per live proc), which walrus rejects
    ("Too many sync wait commands"). Rewrite every instruction carrying
    k>1 waits into (k-1) same-engine NoOps carrying one wait each."""
    for fn in nc.m.functions:
        for bb in fn.blocks:
            new = []
            for inst in bb.instructions:
                si = inst.sync_info
                ow = list(si.on_wait) if si else []
                if len(ow) > 1:
                    for jj, w in enumerate(ow[:-1]):
                        nop = mybir.InstNoOp(
                            name=f"{inst.name}_sw{jj}", ins=[], outs=[])
                        nop.engine = inst.engine
                        nop.sync_info = mybir.SyncInfo(
                            on_wait=[w], on_update=[])
                        new.append(nop)
                    inst.sync_info = mybir.SyncInfo(
                        on_wait=[ow[-1]], on_update=list(si.on_update))
                new.append(inst)
            bb.instructions = new

F32 = mybir.dt.float32
F32R = mybir.dt.float32r
BF16 = mybir.dt.bfloat16

B, L, D = 16, 2048, 64
QC = 512                 # query-chunk (free dim of both matmuls)
NQCHUNK = L // QC        # 4 chunks per batch
KT = 128                 # k rows per tile
N_CORES = 8
N_SLOTS = (B * NQCHUNK) // N_CORES   # 8 units per core
GROUP = 3                # k-tiles per ACT group (2 PSUM s-tiles x 3 banks)
BLOCK = 6                # k-tiles per kv DMA block (3 pairs)
PAIR_W = 65 + 65 + 128   # staged pair width in bf16: [V0|V1|K01]
BLOCK_W = (BLOCK // 2) * PAIR_W


def _schedule(valid_lens):
    """Snake-assign 64 units to 8 slots x 8 cores. Returns (N_list, assign)
    where assign[core][slot] = (batch, qchunk) and N_list[slot] = tile
    count every core runs for that slot."""
    evl = np.where(valid_lens > 0, valid_lens, L).astype(np.int64)
    cost = np.ceil(evl / KT).astype(np.int64)        # per batch
    units = [(int(cost[b]), b, qc) for b in range(B) for qc in range(NQCHUNK)]
    units.sort(key=lambda t: (t[0], t[1], t[2]))
    N_list = []
    assign = [[None] * N_SLOTS for _ in range(N_CORES)]
    for j in range(N_SLOTS):
        grp = units[j * N_CORES:(j + 1) * N_CORES]
        N_list.append(grp[-1][0])
        for c in range(N_CORES):
            _, b, qc = grp[c]
            assign[c][j] = (b, qc)
    return N_list, assign


_PROGRAM_CACHE = {}


def _build_program(N_list):
    key = tuple(N_list)
    if key in _PROGRAM_CACHE:
        return _PROGRAM_CACHE[key]
    n_blocks = [int(np.ceil(n / BLOCK)) for n in N_list]
    TB = int(sum(n_blocks))
    boff = [0]
    for g in n_blocks:
        boff.append(boff[-1] + g)

    nc = bass.Bass()
    kv_d = nc.declare_dram_parameter("kv", [TB, KT, BLOCK_W], BF16,
                                     isOutput=False)
    qT_d = nc.declare_dram_parameter("qT", [N_SLOTS, KT, QC], BF16,
                                     isOutput=False)
    o_d = nc.declare_dram_parameter("o", [N_SLOTS, 65, QC], BF16,
                                    isOutput=True)

    with tile.TileContext(nc) as tc:
        with (
            tc.tile_pool(name="kv_pool", bufs=15) as kv_pool,
            tc.tile_pool(name="q_pool", bufs=1) as q_pool,
            tc.tile_pool(name="p_pool", bufs=6) as p_pool,
            tc.tile_pool(name="ep_pool", bufs=3) as ep_pool,
            tc.tile_pool(name="warm_pool", bufs=1) as warm_pool,
            tc.tile_pool(name="s_pool", bufs=2, space="PSUM") as s_pool,
            tc.tile_pool(name="pv_pool", bufs=2, space="PSUM") as pv_pool,
        ):
            # ACT exp-table warm-up: overlap the one-time table load with
            # the first DMAs instead of stalling the first real group.
            warm = warm_pool.tile([1, 1], F32)
            nc.vector.memset(warm, 0.0)
            nc.scalar.activation(warm, warm, mybir.ActivationFunctionType.Exp)

            # All DMA dispatches are ~630ns each and serialize on the Sync
            # queue, so order them by need: slot 0's kv block and Q^T chunk
            # first (they gate the first matmul), then the rest interleaved
            # round-robin. kv_pool bufs cover every block, so the whole kv
            # stream prefetches upfront with no reuse hazard.
            qt_all = q_pool.tile([KT, N_SLOTS, QC], BF16)
            kv_tiles = {}
            dispatch = []           # (kind, args) in sync-queue order
            for j in range(N_SLOTS):
                for bi in range(int(np.ceil(N_list[j] / BLOCK))):
                    used = min(N_list[j] - bi * BLOCK, BLOCK)
                    dispatch.append(("kv", j, bi, ((used + 1) // 2) * PAIR_W))
                if j + 1 < N_SLOTS:
                    dispatch.append(("qt", j + 1))
            # slot 0's Q^T and kv block are split by partition halves
            # across two DMA rings: the first S matmul (even parity) only
            # needs partitions 0-63 of each, so it starts ~1us earlier.
            _, _, _, w0 = dispatch[0]
            kvb0 = kv_pool.tile([KT, BLOCK_W], BF16, tag="kv")
            kv_tiles[(0, 0)] = kvb0
            for h in (0, 1):
                nc.sync.dma_start(
                    out=qt_all[64 * h:64 * h + 64, 0, :],
                    in_=bass.AP(tensor=qT_d, offset=64 * h * QC,
                                ap=[[QC, 64], [1, QC]]))
                nc.sync.dma_start(
                    out=kvb0[64 * h:64 * h + 64, 0:w0],
                    in_=bass.AP(tensor=kv_d, offset=64 * h * BLOCK_W,
                                ap=[[BLOCK_W, 64], [1, w0]]))
            for item in dispatch[1:]:
                if item[0] == "kv":
                    _, j, bi, w = item
                    kvb = kv_pool.tile([KT, BLOCK_W], BF16, tag="kv")
                    kv_tiles[(j, bi)] = kvb
                    nc.sync.dma_start(
                        out=kvb[:, 0:w],
                        in_=kv_d[boff[j] + bi][:, 0:w])
                else:
                    jq = item[1]
                    nc.sync.dma_start(
                        out=qt_all[:, jq, :],
                        in_=bass.AP(tensor=qT_d, offset=jq * KT * QC,
                                    ap=[[QC, KT], [1, QC]]))

            # software pipeline: PV matmuls of group g are emitted ~2
            # groups behind the S matmuls, and only at PAIR boundaries so
            # S pairs stay adjacent in the in-order PE queue. Each slot
            # accumulates into TWO alternating PSUM banks (pv_a/pv_b) so
            # consecutive PV matmuls hit different banks and the ~128-cycle
            # array drain of one overlaps the stream of the next; the DVE
            # adds the two banks in the epilogue.
            PIPE_DEPTH = 3
            pending = []       # [(pv_ops, j), ...] one entry per group
            epilogues = []     # (j, pv_a, pv_b, n) awaiting PV completion

            def flush_one():
                ops, _ = pending.pop(0)
                for (pv, lhsT, rhs, start, stop) in ops:
                    nc.tensor.matmul(pv, lhsT=lhsT, rhs=rhs,
                                     start=start, stop=stop)

            def emit_epilogues():
                # slot j's pv may be read out once all its PV groups have
                # been flushed (pending is ordered by emission).
                while epilogues and (not pending
                                     or epilogues[0][0] < pending[0][1]):
                    j, pv, n = epilogues.pop(0)
                    pvc = ep_pool.tile([65, QC], BF16, tag="pvc")
                    nc.vector.tensor_copy(pvc, pv)
                    nc.sync.dma_start(out=o_d[j], in_=pvc)

            for j in range(N_SLOTS):
                n = N_list[j]
                pv = pv_pool.tile([65, QC], F32, tag="pv")
                s = None
                group_ops = []
                kvb = None
                for t in range(0, n, 2):
                    if t % BLOCK == 0:
                        kvb = kv_tiles[(j, t // BLOCK)]
                    for u in (t, t + 1):
                        if u >= n:
                            break
                        po = ((u % BLOCK) // 2) * PAIR_W
                        half = 64 * (u % 2)
                        if u % GROUP == 0:
                            s = s_pool.tile([KT, GROUP * QC], F32, tag="s")
                        # S: K^T_u (stationary, array half by parity) @ Q^T
                        nc.tensor.matmul(
                            s[:, (u % GROUP) * QC:(u % GROUP + 1) * QC],
                            lhsT=kvb[half:half + D, po + 130:po + 258],
                            rhs=qt_all[half:half + D, j, :],
                            start=True, stop=True)
                        group_ops.append(
                            (kvb[:, po + 65 * (u % 2):po + 65 * (u % 2) + 65],
                             u))
                        if u % GROUP == GROUP - 1 or u == n - 1:
                            # group complete -> exp on ACT queue (may land
                            # mid-pair; not a PE instruction)
                            g = len(group_ops)
                            p = p_pool.tile([KT, GROUP * QC], BF16, tag="p")
                            nc.scalar.activation(
                                p[:, 0:g * QC], s[:, 0:g * QC],
                                mybir.ActivationFunctionType.Exp, scale=0.125)
                            pending.append(([
                                (pv, vap, p[:, i * QC:(i + 1) * QC],
                                 uu == 0, uu == n - 1)
                                for i, (vap, uu) in enumerate(group_ops)], j))
                            group_ops = []
                    # pair boundary: drain the PV pipeline
                    while len(pending) > PIPE_DEPTH:
                        flush_one()
                        emit_epilogues()
                epilogues.append((j, pv, n))
            while pending:
                flush_one()
                emit_epilogues()
            emit_epilogues()

    split_multi_waits(nc)
    _PROGRAM_CACHE[key] = (nc, boff)
    return nc, boff


def _stage_inputs(queries, keys, values, valid_lens, N_list, assign, boff):
    import ml_dtypes
    bf16 = ml_dtypes.bfloat16
    evl = np.where(valid_lens > 0, valid_lens, L).astype(np.int64)
    zero_q = valid_lens <= 0
    TB = boff[-1]

    # Per-batch precomputed host tensors
    kT_bf = np.ascontiguousarray(keys.transpose(0, 2, 1)).astype(bf16)
    vmask = (np.arange(L)[None, :] < evl[:, None])             # [B, L]
    vp = np.concatenate(
        [values, np.ones((B, L, 1), np.float32)], axis=2)      # [B, L, 65]
    vp_bf = (vp * vmask[:, :, None].astype(np.float32)).astype(bf16)

    in_maps = []
    for c in range(N_CORES):
        kv = np.zeros((TB, KT, BLOCK_W), bf16)
        kvv = kv.reshape(TB, KT, BLOCK // 2, PAIR_W)
        qT = np.zeros((N_SLOTS, KT, QC), bf16)
        for j in range(N_SLOTS):
            b, qc = assign[c][j]
            if not zero_q[b]:
                qT[j, 0:D] = queries[b, qc * QC:(qc + 1) * QC, :].T.astype(bf16)
                qT[j, D:] = qT[j, 0:D]
            n = min(int(np.ceil(evl[b] / KT)), N_list[j])
            for t in range(n):
                bj = boff[j] + t // BLOCK
                pr = (t % BLOCK) // 2
                par = t % 2
                kvv[bj, :, pr, 65 * par:65 * par + 65] = \
                    vp_bf[b, t * KT:(t + 1) * KT, :]
                kvv[bj, 64 * par:64 * par + D, pr, 130:258] = \
                    kT_bf[b, :, t * KT:(t + 1) * KT]
        in_maps.append({"kv": kv, "qT": qT})
    return in_maps


def _gather(results, assign):
    out = np.empty((B, L, D), np.float32)
    for c in range(N_CORES):
        o = results[c]["o"]                       # [N_SLOTS, 65, QC]
        for j in range(N_SLOTS):
            b, qc = assign[c][j]
            of = o[j].astype(np.float32)
            out[b, qc * QC:(qc + 1) * QC, :] = (of[0:D] / of[D:]).T
    return out


def run(queries, keys, values, valid_lens, trace=False):
    queries = np.asarray(queries, np.float32)
    keys = np.asarray(keys, np.float32)
    values = np.asarray(values, np.float32)
    valid_lens = np.asarray(valid_lens)
    N_list, assign = _schedule(valid_lens)
    nc, boff = _build_program(N_list)
    in_maps = _stage_inputs(queries, keys, values, valid_lens, N_list,
                            assign, boff)
    res = run_bass_kernel_spmd(nc, in_maps, list(range(N_CORES)),
                               trace=trace)
    return _gather(res.results, assign), res


def kernel(queries, keys, values, valid_lens):
    out, _ = run(queries, keys, values, valid_lens)
    return out


# revision 19
# speedup vs baseline: 1.0338x; 1.0338x over previous
"""Masked dot-product attention (B=16, Lq=Lk=2048, d=64) on 8 TRN2 NeuronCores.

Distribution
------------
Attention rows are independent, so work is split into 64 units = (batch,
512-query chunk). Unit cost = ceil(valid_len/128) k-tiles; fully-masked
k-tiles contribute exactly zero and are skipped. Units are sorted by cost
(ascending) and snake-assigned to 8 slots x 8 cores; each slot's tile
count is the max within the slot, so all 8 cores run ONE shared SPMD
program (per-core differences live only in the staged data).

Device math per unit (S^T formulation; softmax over the partition axis):
    s_t[k, q]  = (K^T_t weights) @ Q^T           (PE, bf16 x bf16, PSUM)
    p_t[k, q]  = exp(0.125 * s_t)                (ACT, PSUM->SBUF, bf16)
    pv[v, q]  += V'_t^T @ p_t                    (PE, accumulate over t)
where V'_t = [V rows | ones], with rows >= valid_len zeroed on the host —
this applies the key mask AND computes the softmax denominator l = pv[64]
inside the same matmul. The normalize o = pv[0:64] / pv[64] runs on the
HOST during the gather (device ships raw pv per slot) — no on-device
reciprocal/broadcast chain, so the kernel tail is one copy + one DMA.

v2 layout/schedule changes vs the 62us baseline (trace-driven):
- kv is staged in bf16, PAIR-PACKED: per pair of k-tiles the staged
  region is [V_even(65) | V_odd(65) | K_pair(128)] = 258 bf16 cols,
  where K_pair holds K^T of the even tile in partitions 0-63 and of the
  odd tile in partitions 64-127. This removes the half-partition zero
  padding of the old layout AND halves the bytes: 8.1MB -> 2.9MB HBM
  per core (the old kv stream saturated ~360GB/s for 16us).
- S matmuls are emitted in strictly adjacent (even, odd) pairs that
  occupy disjoint PE array halves; PV batches are only injected at pair
  boundaries, so every pair streams concurrently (~427ns for 2 tiles).
  exp instructions (ACT queue) may land mid-pair — they don't break PE
  queue adjacency.
- Q^T is staged doubled into both partition halves (bf16); slot 0's
  chunk is DMA'd first, slots 1-7 follow in one 0.9MB prefetch.
- All DMAs are dispatched from the Sync queue (HWDGE); ~620ns per
  dispatch, 25 dispatches total.
- Every TPB instruction may carry at most ONE sync wait on this walrus;
  split_multi_waits() post-processes the scheduled program.
"""
import numpy as np

import concourse.bass as bass
import concourse.mybir as mybir
import concourse.tile as tile
from concourse.bass_utils import run_bass_kernel_spmd


def _ensure_ntff_hook():
    """The agent image's `antenv` package lacks `axon_hooks`, so
    run_bass_kernel_spmd(trace=True) dies importing it (trn_boot degrades
    silently). Inject a live module with the get/set pair and register the
    ctypes-driven hook trn_boot would have. No-op if already present."""
    import importlib.util
    import sys
    import types
    try:
        if importlib.util.find_spec("antenv.axon_hooks") is not None:
            return
    except Exception:
        pass
    if "antenv.axon_hooks" in sys.modules:
        return
    mod = types.ModuleType("antenv.axon_hooks")
    mod._hook = None
    mod.set_axon_ntff_profile_hook = lambda h: setattr(mod, "_hook", h)
    mod.get_axon_ntff_profile_hook = lambda: mod._hook
    sys.modules["antenv.axon_hooks"] = mod
    try:
        import antenv
        antenv.axon_hooks = mod
        from trn_agent_boot.trn_boot import _ntff_profile_via_ctypes
        mod._hook = _ntff_profile_via_ctypes("/opt/axon/libaxon_pjrt.so")
    except Exception:
        pass


_ensure_ntff_hook()


def split_multi_waits(nc):
    """TRN2 TPB instructions encode a single sync-wait slot. Tile's
    add_semaphores can emit several waits on one instruction (and the
    kernel-tail drain aggregates one per live proc), which walrus rejects
    ("Too many sync wait commands"). Rewrite every instruction carrying
    k>1 waits into (k-1) same-engine NoOps carrying one wait each."""
    for fn in nc.m.functions:
        for bb in fn.blocks:
            new = []
            for inst in bb.instructions:
                si = inst.sync_info
                ow = list(si.on_wait) if si else []
                if len(ow) > 1:
                    for jj, w in enumerate(ow[:-1]):
                        nop = mybir.InstNoOp(
                            name=f"{inst.name}_sw{jj}", ins=[], outs=[])
                        nop.engine = inst.engine
                        nop.sync_info = mybir.SyncInfo(
                            on_wait=[w], on_update=[])
                        new.append(nop)
                    inst.sync_info = mybir.SyncInfo(
                        on_wait=[ow[-1]], on_update=list(si.on_update))
                new.append(inst)
            bb.instructions = new

F32 = mybir.dt.float32
F32R = mybir.dt.float32r
BF16 = mybir.dt.bfloat16

B, L, D = 16, 2048, 64
QC = 512                 # query-chunk (free dim of both matmuls)
NQCHUNK = L // QC        # 4 chunks per batch
KT = 128                 # k rows per tile
N_CORES = 8
N_SLOTS = (B * NQCHUNK) // N_CORES   # 8 units per core
GROUP = 3                # k-tiles per ACT group (2 PSUM s-tiles x 3 banks)
BLOCK = 6                # k-tiles per kv DMA block (3 pairs)
PAIR_W = 65 + 65 + 128   # staged pair width in bf16: [V0|V1|K01]
BLOCK_W = (BLOCK // 2) * PAIR_W


def _schedule(valid_lens):
    """Snake-assign 64 units to 8 slots x 8 cores. Returns (N_list, assign)
    where assign[core][slot] = (batch, qchunk) and N_list[slot] = tile
    count every core runs for that slot."""
    evl = np.where(valid_lens > 0, valid_lens, L).astype(np.int64)
    cost = np.ceil(evl / KT).astype(np.int64)        # per batch
    units = [(int(cost[b]), b, qc) for b in range(B) for qc in range(NQCHUNK)]
    units.sort(key=lambda t: (t[0], t[1], t[2]))
    N_list = []
    assign = [[None] * N_SLOTS for _ in range(N_CORES)]
    for j in range(N_SLOTS):
        grp = units[j * N_CORES:(j + 1) * N_CORES]
        N_list.append(grp[-1][0])
        for c in range(N_CORES):
            _, b, qc = grp[c]
            assign[c][j] = (b, qc)
    return N_list, assign


_PROGRAM_CACHE = {}


def _build_program(N_list):
    key = tuple(N_list)
    if key in _PROGRAM_CACHE:
        return _PROGRAM_CACHE[key]
    n_blocks = [int(np.ceil(n / BLOCK)) for n in N_list]
    TB = int(sum(n_blocks))
    boff = [0]
    for g in n_blocks:
        boff.append(boff[-1] + g)

    nc = bass.Bass()
    kv_d = nc.declare_dram_parameter("kv", [TB, KT, BLOCK_W], BF16,
                                     isOutput=False)
    qT_d = nc.declare_dram_parameter("qT", [N_SLOTS, KT, QC], BF16,
                                     isOutput=False)
    o_d = nc.declare_dram_parameter("o", [N_SLOTS, 65, QC], BF16,
                                    isOutput=True)

    with tile.TileContext(nc) as tc:
        with (
            tc.tile_pool(name="kv_pool", bufs=15) as kv_pool,
            tc.tile_pool(name="q_pool", bufs=1) as q_pool,
            tc.tile_pool(name="p_pool", bufs=6) as p_pool,
            tc.tile_pool(name="ep_pool", bufs=3) as ep_pool,
            tc.tile_pool(name="warm_pool", bufs=1) as warm_pool,
            tc.tile_pool(name="s_pool", bufs=2, space="PSUM") as s_pool,
            tc.tile_pool(name="pv_pool", bufs=2, space="PSUM") as pv_pool,
        ):
            # ACT exp-table warm-up: overlap the one-time table load with
            # the first DMAs instead of stalling the first real group.
            warm = warm_pool.tile([1, 1], F32)
            nc.vector.memset(warm, 0.0)
            nc.scalar.activation(warm, warm, mybir.ActivationFunctionType.Exp)

            # All DMA dispatches are ~630ns each and serialize on the Sync
            # queue, so order them by need: slot 0's kv block and Q^T chunk
            # first (they gate the first matmul), then the rest interleaved
            # round-robin. kv_pool bufs cover every block, so the whole kv
            # stream prefetches upfront with no reuse hazard.
            qt_all = q_pool.tile([KT, N_SLOTS, QC], BF16)
            kv_tiles = {}
            dispatch = []           # (kind, args) in sync-queue order
            for j in range(N_SLOTS):
                for bi in range(int(np.ceil(N_list[j] / BLOCK))):
                    used = min(N_list[j] - bi * BLOCK, BLOCK)
                    dispatch.append(("kv", j, bi, ((used + 1) // 2) * PAIR_W))
                if j + 1 < N_SLOTS:
                    dispatch.append(("qt", j + 1))
            order = [("qt", 0), dispatch[0]] + dispatch[1:]
            for item in order:
                if item[0] == "kv":
                    _, j, bi, w = item
                    kvb = kv_pool.tile([KT, BLOCK_W], BF16, tag="kv")
                    kv_tiles[(j, bi)] = kvb
                    nc.sync.dma_start(
                        out=kvb[:, 0:w],
                        in_=kv_d[boff[j] + bi][:, 0:w])
                else:
                    jq = item[1]
                    nc.sync.dma_start(
                        out=qt_all[:, jq, :],
                        in_=bass.AP(tensor=qT_d, offset=jq * KT * QC,
                                    ap=[[QC, KT], [1, QC]]))

            # software pipeline: PV matmuls of group g are emitted ~2
            # groups behind the S matmuls, and only at PAIR boundaries so
            # S pairs stay adjacent in the in-order PE queue. Each slot
            # accumulates into TWO alternating PSUM banks (pv_a/pv_b) so
            # consecutive PV matmuls hit different banks and the ~128-cycle
            # array drain of one overlaps the stream of the next; the DVE
            # adds the two banks in the epilogue.
            PIPE_DEPTH = 3
            pending = []       # [(pv_ops, j), ...] one entry per group
            epilogues = []     # (j, pv_a, pv_b, n) awaiting PV completion

            def flush_one():
                ops, _ = pending.pop(0)
                for (pv, lhsT, rhs, start, stop) in ops:
                    nc.tensor.matmul(pv, lhsT=lhsT, rhs=rhs,
                                     start=start, stop=stop)

            def emit_epilogues():
                # slot j's pv may be read out once all its PV groups have
                # been flushed (pending is ordered by emission).
                while epilogues and (not pending
                                     or epilogues[0][0] < pending[0][1]):
                    j, pv, n = epilogues.pop(0)
                    pvc = ep_pool.tile([65, QC], BF16, tag="pvc")
                    nc.vector.tensor_copy(pvc, pv)
                    nc.sync.dma_start(out=o_d[j], in_=pvc)

            for j in range(N_SLOTS):
                n = N_list[j]
                pv = pv_pool.tile([65, QC], F32, tag="pv")
                s = None
                group_ops = []
                kvb = None
                for t in range(0, n, 2):
                    if t % BLOCK == 0:
                        kvb = kv_tiles[(j, t // BLOCK)]
                    for u in (t, t + 1):
                        if u >= n:
                            break
                        po = ((u % BLOCK) // 2) * PAIR_W
                        half = 64 * (u % 2)
                        if u % GROUP == 0:
                            s = s_pool.tile([KT, GROUP * QC], F32, tag="s")
                        # S: K^T_u (stationary, array half by parity) @ Q^T
                        nc.tensor.matmul(
                            s[:, (u % GROUP) * QC:(u % GROUP + 1) * QC],
                            lhsT=kvb[half:half + D, po + 130:po + 258],
                            rhs=qt_all[half:half + D, j, :],
                            start=True, stop=True)
                        group_ops.append(
                            (kvb[:, po + 65 * (u % 2):po + 65 * (u % 2) + 65],
                             u))
                        if u % GROUP == GROUP - 1 or u == n - 1:
                            # group complete -> exp on ACT queue (may land
                            # mid-pair; not a PE instruction)
                            g = len(group_ops)
                            p = p_pool.tile([KT, GROUP * QC], BF16, tag="p")
                            nc.scalar.activation(
                                p[:, 0:g * QC], s[:, 0:g * QC],
                                mybir.ActivationFunctionType.Exp, scale=0.125)
                            pending.append(([
                                (pv, vap, p[:, i * QC:(i + 1) * QC],
                                 uu == 0, uu == n - 1)
                                for i, (vap, uu) in enumerate(group_ops)], j))
                            group_ops = []
                    # pair boundary: drain the PV pipeline
                    while len(pending) > PIPE_DEPTH:
                        flush_one()
                        emit_epilogues()
                epilogues.append((j, pv, n))
            while pending:
                flush_one()
                emit_epilogues()
            emit_epilogues()

    split_multi_waits(nc)
    _PROGRAM_CACHE[key] = (nc, boff)
    return nc, boff


def _stage_inputs(queries, keys, values, valid_lens, N_list, assign, boff):
    import ml_dtypes
    bf16 = ml_dtypes.bfloat16
    evl = np.where(valid_lens > 0, valid_lens, L).astype(np.int64)
    zero_q = valid_lens <= 0
    TB = boff[-1]

    # Per-batch precomputed host tensors
    kT_bf = np.ascontiguousarray(keys.transpose(0, 2, 1)).astype(bf16)
    vmask = (np.arange(L)[None, :] < evl[:, None])             # [B, L]
    vp = np.concatenate(
        [values, np.ones((B, L, 1), np.float32)], axis=2)      # [B, L, 65]
    vp_bf = (vp * vmask[:, :, None].astype(np.float32)).astype(bf16)

    in_maps = []
    for c in range(N_CORES):
        kv = np.zeros((TB, KT, BLOCK_W), bf16)
        kvv = kv.reshape(TB, KT, BLOCK // 2, PAIR_W)
        qT = np.zeros((N_SLOTS, KT, QC), bf16)
        for j in range(N_SLOTS):
            b, qc = assign[c][j]
            if not zero_q[b]:
                qT[j, 0:D] = queries[b, qc * QC:(qc + 1) * QC, :].T.astype(bf16)
                qT[j, D:] = qT[j, 0:D]
            n = min(int(np.ceil(evl[b] / KT)), N_list[j])
            for t in range(n):
                bj = boff[j] + t // BLOCK
                pr = (t % BLOCK) // 2
                par = t % 2
                kvv[bj, :, pr, 65 * par:65 * par + 65] = \
                    vp_bf[b, t * KT:(t + 1) * KT, :]
                kvv[bj, 64 * par:64 * par + D, pr, 130:258] = \
                    kT_bf[b, :, t * KT:(t + 1) * KT]
        in_maps.append({"kv": kv, "qT": qT})
    return in_maps


def _gather(results, assign):
    out = np.empty((B, L, D), np.float32)
    for c in range(N_CORES):
        o = results[c]["o"]                       # [N_SLOTS, 65, QC]
        for j in range(N_SLOTS):
            b, qc = assign[c][j]
            of = o[j].astype(np.float32)
            out[b, qc * QC:(qc + 1) * QC, :] = (of[0:D] / of[D:]).T
    return out


def run(queries, keys, values, valid_lens, trace=False):
    queries = np.asarray(queries, np.float32)
    keys = np.asarray(keys, np.float32)
    values = np.asarray(values, np.float32)
    valid_lens = np.asarray(valid_lens)
    N_list, assign = _schedule(valid_lens)
    nc, boff = _build_program(N_list)
    in_maps = _stage_inputs(queries, keys, values, valid_lens, N_list,
                            assign, boff)
    res = run_bass_kernel_spmd(nc, in_maps, list(range(N_CORES)),
                               trace=trace)
    return _gather(res.results, assign), res


def kernel(queries, keys, values, valid_lens):
    out, _ = run(queries, keys, values, valid_lens)
    return out
